# revision 20
# baseline (speedup 1.0000x reference)
"""Bass/Trainium2 kernel for nn_EntityLabeler (LSTM+CRF NLL loss).

Contract: kernel(**inputs) takes FULL unsharded inputs (as produced by
setup_inputs) and returns the FULL scalar loss. Internally shards the
batch (128 rows) across 8 NeuronCores (16 rows each), computes a partial
loss per core on-device, and sums the 8 partials on the host.

Device algorithm per core (all layouts transposed: feature-on-partition,
batch-on-free):
  1. Embedding gather (indirect DMA) in 32-step chunks -> PE transpose ->
     x.T tiles (bf16).
  2. Input projection xp.T = W_ih @ x.T + b (matmul, bf16) into an SBUF
     ring, packed per-step as [i0 i1 f0 f1 o0 o1 g0 g1] x 16 batch.
  3. LSTM recurrence: per step one identity-matmul injects xp into PSUM,
     then 16 accumulate matmuls (W_hh.T stationary, h.T moving) produce
     gates.T [128, 128]; sigmoid/tanh on ACT, cell update on DVE.
  4. Emissions em.T = W_lin @ relu(h.T) + b_lin via matmul (b_lin folded
     in as a K=1 matmul row).
  5. CRF log-partition via exp-domain linear scan:
     p <- (ET.T @ p) * exp(em_t), renormalized every 8 steps;
     logZ = sum(log s) + log(sum p*exp(end)).
  6. Gold-path score via one-hot matmuls (L=9).
"""

import sys
from contextlib import ExitStack

import numpy as np

for _p in ("/opt/trn_rl_repo",):
    if _p not in sys.path:
        sys.path.insert(0, _p)

import concourse.bass as bass
import concourse.bacc as bacc
import concourse.tile as tile
from concourse import mybir
from concourse.masks import make_identity
from concourse.bass_utils import run_bass_kernel_spmd

F32 = mybir.dt.float32
BF16 = mybir.dt.bfloat16
I32 = mybir.dt.int32
AF = mybir.ActivationFunctionType
OP = mybir.AluOpType

B, S, V, E, H, L = 128, 512, 32000, 256, 256, 9
NCORES = 8
BL = B // NCORES           # 16 batch rows per core
G4 = 4 * H                 # 1024 gate units
CH = 32                    # LSTM steps per chunk
NORM_EVERY = 8

# column offset of each (gate, half) region inside the per-step [128, 128]
# gates.T PSUM tile / xp ring block. gate order (torch): i=0, f=1, g=2, o=3.
POS = {(0, 0): 0, (0, 1): 16, (1, 0): 32, (1, 1): 48,
       (3, 0): 64, (3, 1): 80, (2, 0): 96, (2, 1): 112}


def build_program(n_steps: int = S, debug: bool = False):
    """Emit the full Bass/Tile program for one core. Returns nc."""
    assert n_steps % CH == 0
    nchunk = n_steps // CH
    ctok = CH * BL                     # tokens per chunk (512)
    tok = n_steps * BL
    n_norm = (n_steps - 1) // NORM_EVERY       # renormalizations in scan
    sall_w = (n_norm + 1) * BL                 # log-factors incl. final z

    nc = bacc.Bacc("TRN2", target_bir_lowering=False)

    # ---- DRAM I/O ----
    emb_d = nc.dram_tensor("emb", [V, E], F32, kind="ExternalInput")
    idx_d = nc.dram_tensor("idx", [tok, 1], I32, kind="ExternalInput")
    labT_d = nc.dram_tensor("labT", [n_steps, BL], I32, kind="ExternalInput")
    # all weights in one array (single DMA -> single wait for consumers):
    # cols [0:1024] wihT k0, [1024:2048] wihT k1, [2048:3072] whhT k0,
    # [3072:4096] whhT k1, [4096:4105] wlinT k0, [4105:4114] wlinT k1
    wpack_d = nc.dram_tensor("wpack", [128, 4114], F32, kind="ExternalInput")
    # small constants in one array: cols [0:8] bihT, [8:16] bhhT,
    # [16] stT, [17] enT, [18:27] trans, [27:36] blin row (partition 0)
    spack_d = nc.dram_tensor("spack", [128, 36], F32, kind="ExternalInput")

    loss_d = nc.dram_tensor("loss", [1, 1], F32, kind="ExternalOutput")
    if debug:
        score_d = nc.dram_tensor("score", [1, BL], F32, kind="ExternalOutput")
        logz_d = nc.dram_tensor("logz", [1, BL], F32, kind="ExternalOutput")

    with tile.TileContext(nc) as tc, ExitStack() as ctx:
        cst = ctx.enter_context(tc.tile_pool(name="cst", bufs=1))
        stage = ctx.enter_context(tc.tile_pool(name="stage", bufs=2))
        big = ctx.enter_context(tc.tile_pool(name="bigbuf", bufs=1))
        xgp = ctx.enter_context(tc.tile_pool(name="xgp", bufs=6))
        xtp = ctx.enter_context(tc.tile_pool(name="xtp", bufs=4))
        xpr = ctx.enter_context(tc.tile_pool(name="xpr", bufs=2))
        hcp = ctx.enter_context(tc.tile_pool(name="hcp", bufs=3))
        gat = ctx.enter_context(tc.tile_pool(name="gat", bufs=4))
        sml = ctx.enter_context(tc.tile_pool(name="sml", bufs=6))
        scn = ctx.enter_context(tc.tile_pool(name="scn", bufs=6))
        psA = ctx.enter_context(tc.tile_pool(name="psA", bufs=4, space="PSUM"))
        psB = ctx.enter_context(tc.tile_pool(name="psB", bufs=2, space="PSUM"))
        psC = ctx.enter_context(tc.tile_pool(name="psC", bufs=2, space="PSUM"))

        # ---------- constants / weights ----------
        id_bf = cst.tile([128, 128], BF16, tag="id_bf")
        make_identity(nc, id_bf[:, :])
        id_f32 = cst.tile([128, 128], F32, tag="id_f32")
        make_identity(nc, id_f32[:, :])

        warm_ps = psC.tile([1, 1], F32, tag="psC", name="warm_ps")
        nc.tensor.matmul(warm_ps[:, :], lhsT=id_f32[:, 0:1],
                         rhs=id_f32[:, 0:1], start=True, stop=True)

        wpk = cst.tile([128, 4114], F32, tag="wpk")
        nc.sync.dma_start(out=wpk[:, :], in_=wpack_d[:, :])
        spk = cst.tile([128, 36], F32, tag="spk")
        nc.sync.dma_start(out=spk[:, :], in_=spack_d[:, :])

        def cast_bf(src_ap, n_m, tag):
            bf_t = cst.tile([128, n_m], BF16, tag=tag)
            nc.vector.tensor_copy(bf_t[:, :], src_ap)
            return bf_t

        wih_bf = [cast_bf(wpk[:, c * 1024:(c + 1) * 1024], 1024, f"wih{c}")
                  for c in range(2)]
        whh_bf = [cast_bf(wpk[:, 2048 + c * 1024: 2048 + (c + 1) * 1024],
                          1024, f"whh{c}") for c in range(2)]
        wlin_bf = [cast_bf(wpk[:, 4096 + c * L: 4096 + (c + 1) * L], L,
                           f"wlin{c}") for c in range(2)]

        bsum = cst.tile([128, 8], F32, tag="bsum")
        nc.vector.tensor_add(bsum[:, :], spk[:, 0:8], spk[:, 8:16])
        stT = spk[0:L, 16:17]
        enT = spk[0:L, 17:18]
        trans_t = spk[0:L, 18:27]
        blin_bf = cst.tile([1, L], BF16, tag="blinbf")
        nc.vector.tensor_copy(blin_bf[:, :], spk[0:1, 27:36])
        ones_ctok_bf = cst.tile([1, ctok], BF16, tag="onesctok")
        nc.vector.memset(ones_ctok_bf[:, :], 1.0)

        expSt = cst.tile([L, 1], F32, tag="expSt")
        nc.scalar.activation(expSt[:, :], stT, AF.Exp)
        expEn = cst.tile([L, 1], F32, tag="expEn")
        nc.scalar.activation(expEn[:, :], enT, AF.Exp)
        ET = cst.tile([L, L], F32, tag="ET")
        nc.scalar.activation(ET[:, :], trans_t, AF.Exp)
        ones9 = cst.tile([L, 1], F32, tag="ones9")
        nc.vector.memset(ones9[:, :], 1.0)
        ones1_9 = cst.tile([1, L], F32, tag="ones19")
        nc.vector.memset(ones1_9[:, :], 1.0)

        # ---------- one-hot label matrix OHT [L, tok] ----------
        iota9 = cst.tile([L, 1], I32, tag="iota9")
        nc.gpsimd.iota(iota9[:, :], pattern=[[0, 1]], base=0, channel_multiplier=1)
        iota9f = cst.tile([L, 1], F32, tag="iota9f")
        nc.vector.tensor_copy(iota9f[:, :], iota9[:, :])
        OHT = big.tile([L, tok], F32, tag="OHT")
        lab1 = stage.tile([1, tok], I32, tag="lab1", bufs=1)
        lab_flat = bass.AP(tensor=labT_d, offset=0, ap=[[0, 1], [1, tok]])
        nc.sync.dma_start(out=lab1[:, :], in_=lab_flat)
        lchunk = 512
        for q in range(tok // lchunk):
            sl = slice(q * lchunk, (q + 1) * lchunk)
            labf1 = stage.tile([1, lchunk], F32, tag="labf1")
            nc.vector.tensor_copy(labf1[:, :], lab1[:, sl])
            lab_ps = psC.tile([L, lchunk], F32, tag="psC", name="lab_ps")
            nc.tensor.matmul(lab_ps[:, :], lhsT=ones1_9[:, :],
                             rhs=labf1[:, :], start=True, stop=True)
            labrep = stage.tile([L, lchunk], F32, tag="labrep")
            nc.vector.tensor_copy(labrep[:, :], lab_ps[:, :])
            nc.vector.tensor_scalar(
                out=OHT[:, sl], in0=labrep[:, :],
                scalar1=iota9f[:, :], scalar2=None, op0=OP.is_equal)

        # ---------- big persistent buffers ----------
        EE = big.tile([L, tok], F32, tag="EE")          # exp(emissions.T)
        sall = big.tile([1, sall_w], F32, tag="sall")   # scan log-factors
        etsum = cst.tile([1, BL], F32, tag="etsum")     # sum_t em[lab] per b
        nc.vector.memset(etsum[:, :], 0.0)

        # all gather indices in one DMA: idx_all[p, g] = idx[g*128 + p]
        idx_all = cst.tile([128, tok // 128], I32, tag="idx_all")
        idx_ap = bass.AP(tensor=idx_d, offset=0,
                         ap=[[1, 128], [128, tok // 128]])
        nc.sync.dma_start(out=idx_all[:, :], in_=idx_ap)

        # ---------- main chunk pipeline ----------
        cstate = cst.tile([128, 32], F32, tag="cstate")  # c.T both halves
        h_prev = None        # AP of previous step's h.T [128, 32] (bf16)
        hT_chunks = []

        for k in range(nchunk):
            # -- gather 512 tokens & transpose to x.T (bf16) --
            xT = [xtp.tile([128, ctok], BF16, tag="xT", name=f"xT{ec}")
                  for ec in range(2)]
            for q in range(4):
                g = k * 4 + q
                xg = xgp.tile([128, E], F32, tag="xg")
                nc.gpsimd.indirect_dma_start(
                    out=xg[:, :], out_offset=None,
                    in_=emb_d[:, :],
                    in_offset=bass.IndirectOffsetOnAxis(
                        ap=idx_all[:, g:g + 1], axis=0))
                for ec in range(2):
                    tp = psA.tile([128, 128], F32, tag="psA")
                    nc.tensor.transpose(
                        tp[:, :], xg[:, ec * 128:(ec + 1) * 128], id_f32[:, :])
                    dst = xT[ec][:, q * 128:(q + 1) * 128]
                    nc.vector.tensor_copy(dst, tp[:, :])

            # -- input projection xp ring for this chunk --
            xpring = xpr.tile([128, CH * 128], BF16, tag="xpring")
            xpv = xpring.rearrange("p (t g) -> p t g", g=128)
            for gi, half in ((0, 0), (0, 1), (1, 0), (1, 1),
                             (3, 0), (3, 1), (2, 0), (2, 1)):
                j = gi * 2 + half
                xp_ps = psB.tile([128, ctok], F32, tag="psB")
                for c in range(2):
                    nc.tensor.matmul(
                        xp_ps[:, :],
                        lhsT=wih_bf[c][:, j * 128:(j + 1) * 128],
                        rhs=xT[c][:, :], start=(c == 0), stop=(c == 1))
                src = xp_ps.rearrange("p (t b) -> p t b", b=BL)
                dst = xpv[:, :, POS[(gi, half)]:POS[(gi, half)] + BL]
                nc.scalar.add(dst, src, add=bsum[:, j:j + 1])

            # sync DVE's view of ACT's xpring writes (keeps every
            # consumer at <=1 semaphore wait; walrus ISA limit)
            sync_j = sml.tile([128, 1], BF16, tag="syncj")
            nc.vector.tensor_copy(sync_j[:, :], xpring[:, 0:1])

            # -- LSTM recurrence over this chunk --
            hT = hcp.tile([128, CH * 32], BF16, tag="hT")
            hT_chunks.append(hT)
            for tl in range(CH):
                t = k * CH + tl
                if t == 0:
                    # h == 0: gates are just the input projection
                    gpre_i = xpv[:, 0, 0:96]
                    gpre_g = xpv[:, 0, 96:128]
                else:
                    ps = psA.tile([128, 96], F32, tag="psA", name="ps_ifo")
                    ps_g = psA.tile([128, 32], F32, tag="psA", name="ps_g")
                    for gi, half in ((2, 0), (2, 1), (0, 0), (0, 1),
                                     (1, 0), (1, 1), (3, 0), (3, 1)):
                        j = gi * 2 + half
                        pos = POS[(gi, half)]
                        dst = (ps_g[:, pos - 96:pos - 96 + BL] if gi == 2
                               else ps[:, pos:pos + BL])
                        for c in range(2):
                            nc.tensor.matmul(
                                dst,
                                lhsT=whh_bf[c][:, j * 128:(j + 1) * 128],
                                rhs=h_prev[:, c * BL:(c + 1) * BL],
                                start=(c == 0), stop=(c == 1))
                    gi_t = gat.tile([128, 96], F32, tag="gprei")
                    nc.vector.tensor_tensor(
                        out=gi_t[:, :], in0=ps[:, :], in1=xpv[:, tl, 0:96],
                        op=OP.add)
                    gg_t = gat.tile([128, 32], F32, tag="gpreg")
                    nc.vector.tensor_tensor(
                        out=gg_t[:, :], in0=ps_g[:, :],
                        in1=xpv[:, tl, 96:128], op=OP.add)
                    gpre_i, gpre_g = gi_t[:, :], gg_t[:, :]
                sifo = gat.tile([128, 96], F32, tag="sifo")
                nc.scalar.activation(sifo[:, :], gpre_i, AF.Sigmoid)
                tg = gat.tile([128, 32], F32, tag="tg")
                nc.scalar.activation(tg[:, :], gpre_g, AF.Tanh)
                if t == 0:
                    nc.vector.tensor_tensor(
                        out=cstate[:, :], in0=sifo[:, 0:32], in1=tg[:, :],
                        op=OP.mult)
                else:
                    fc = sml.tile([128, 32], F32, tag="fc")
                    nc.vector.tensor_tensor(
                        out=fc[:, :], in0=sifo[:, 32:64], in1=cstate[:, :],
                        op=OP.mult)
                    ig = sml.tile([128, 32], F32, tag="ig")
                    nc.vector.tensor_tensor(
                        out=ig[:, :], in0=sifo[:, 0:32], in1=tg[:, :],
                        op=OP.mult)
                    nc.vector.tensor_tensor(
                        out=cstate[:, :], in0=fc[:, :], in1=ig[:, :],
                        op=OP.add)
                tc_t = gat.tile([128, 32], F32, tag="tc")
                nc.scalar.activation(tc_t[:, :], cstate[:, :], AF.Tanh)
                h_slice = hT[:, tl * 32:(tl + 1) * 32]
                nc.vector.tensor_tensor(
                    out=h_slice, in0=sifo[:, 64:96], in1=tc_t[:, :],
                    op=OP.mult)
                h_prev = h_slice

            # -- emissions for this chunk --
            relu_t = xtp.tile([128, CH * 32], BF16, tag="relu")
            nc.scalar.activation(relu_t[:, :], hT[:, :], AF.Relu)
            rv = relu_t.rearrange("p (t s) -> p t s", s=32)
            em_ps = psB.tile([L, ctok], F32, tag="psB")
            for c in range(2):
                nc.tensor.matmul(
                    em_ps[:, :], lhsT=wlin_bf[c][:, :],
                    rhs=rv[:, :, c * BL:(c + 1) * BL],
                    start=(c == 0), stop=False)
            nc.tensor.matmul(
                em_ps[:, :], lhsT=blin_bf[:, :], rhs=ones_ctok_bf[:, :],
                start=False, stop=True)
            nc.scalar.activation(
                EE[:, k * ctok:(k + 1) * ctok], em_ps[:, :], AF.Exp)
            prod = stage.tile([L, ctok], F32, tag="prod")
            nc.vector.tensor_tensor(
                out=prod[:, :], in0=em_ps[:, :],
                in1=OHT[:, k * ctok:(k + 1) * ctok], op=OP.mult)
            et_ps = psC.tile([1, ctok], F32, tag="psC")
            nc.tensor.matmul(et_ps[:, :], lhsT=ones9[:, :], rhs=prod[:, :],
                             start=True, stop=True)
            etc = sml.tile([1, BL], F32, tag="etc")
            nc.vector.tensor_reduce(
                out=etc[:, :], in_=et_ps.rearrange("p (t b) -> p b t", b=BL),
                axis=mybir.AxisListType.X, op=OP.add)
            nc.vector.tensor_tensor(
                out=etsum[:, :], in0=etsum[:, :], in1=etc[:, :], op=OP.add)

        # ---------- numerator: transition scores ----------
        trsum = cst.tile([1, BL], F32, tag="trsum")
        nc.vector.memset(trsum[:, :], 0.0)
        for cc in range(nchunk):
            w = ctok if cc < nchunk - 1 else ctok - BL
            q_ps = psB.tile([L, ctok], F32, tag="psB")
            nc.tensor.matmul(
                q_ps[:, :w], lhsT=trans_t[:, :],
                rhs=OHT[:, cc * ctok: cc * ctok + w], start=True, stop=True)
            tprod = stage.tile([L, ctok], F32, tag="tprod")
            nc.vector.tensor_tensor(
                out=tprod[:, :w], in0=q_ps[:, :w],
                in1=OHT[:, cc * ctok + BL: cc * ctok + BL + w], op=OP.mult)
            tr_ps = psC.tile([1, ctok], F32, tag="psC")
            nc.tensor.matmul(tr_ps[:, :w], lhsT=ones9[:, :], rhs=tprod[:, :w],
                             start=True, stop=True)
            trc = sml.tile([1, BL], F32, tag="trc")
            nc.vector.tensor_reduce(
                out=trc[:, :],
                in_=tr_ps[:, :w].rearrange("p (t b) -> p b t", b=BL),
                axis=mybir.AxisListType.X, op=OP.add)
            nc.vector.tensor_tensor(
                out=trsum[:, :], in0=trsum[:, :], in1=trc[:, :], op=OP.add)

        # start / end scores: weights-as-lhsT does mul+colsum in one matmul
        st_ps = psC.tile([1, BL], F32, tag="psC")
        nc.tensor.matmul(st_ps[:, :], lhsT=stT[:, :], rhs=OHT[:, 0:BL],
                         start=True, stop=True)
        en_ps = psC.tile([1, BL], F32, tag="psC")
        nc.tensor.matmul(en_ps[:, :], lhsT=enT[:, :],
                         rhs=OHT[:, tok - BL:tok], start=True, stop=True)
        score = cst.tile([1, BL], F32, tag="score")
        nc.vector.tensor_copy(score[:, :], st_ps[:, :])
        nc.vector.tensor_tensor(out=score[:, :], in0=score[:, :],
                                in1=en_ps[:, :], op=OP.add)
        nc.vector.tensor_tensor(out=score[:, :], in0=score[:, :],
                                in1=etsum[:, :], op=OP.add)
        nc.vector.tensor_tensor(out=score[:, :], in0=score[:, :],
                                in1=trsum[:, :], op=OP.add)

        # ---------- CRF forward scan (exp domain) ----------
        sync_e = sml.tile([L, 1], F32, tag="synce")
        nc.vector.tensor_copy(sync_e[:, :], EE[:, tok - 1:tok])
        p_cur = scn.tile([L, BL], F32, tag="p")
        nc.vector.tensor_scalar(
            out=p_cur[:, :], in0=EE[:, 0:BL], scalar1=expSt[:, :],
            scalar2=None, op0=OP.mult)
        nidx = 0
        for t in range(1, n_steps):
            q_ps = psC.tile([L, BL], F32, tag="psC")
            nc.tensor.matmul(q_ps[:, :], lhsT=ET[:, :], rhs=p_cur[:, :],
                             start=True, stop=True)
            p_new = scn.tile([L, BL], F32, tag="p")
            nc.vector.tensor_tensor(
                out=p_new[:, :], in0=q_ps[:, :],
                in1=EE[:, t * BL:(t + 1) * BL], op=OP.mult)
            p_cur = p_new
            if t % NORM_EVERY == 0:
                s_ps = psC.tile([1, BL], F32, tag="psC")
                nc.tensor.matmul(s_ps[:, :], lhsT=ones9[:, :],
                                 rhs=p_cur[:, :], start=True, stop=True)
                nc.vector.tensor_copy(
                    sall[:, nidx * BL:(nidx + 1) * BL], s_ps[:, :])
                rs = scn.tile([1, BL], F32, tag="rs")
                nc.vector.reciprocal(rs[:, :], s_ps[:, :])
                bc_ps = psC.tile([L, BL], F32, tag="psC")
                nc.tensor.matmul(bc_ps[:, :], lhsT=ones1_9[:, :],
                                 rhs=rs[:, :], start=True, stop=True)
                p_new2 = scn.tile([L, BL], F32, tag="p")
                nc.vector.tensor_tensor(
                    out=p_new2[:, :], in0=p_cur[:, :], in1=bc_ps[:, :],
                    op=OP.mult)
                p_cur = p_new2
                nidx += 1
        pe = scn.tile([L, BL], F32, tag="pe")
        nc.vector.tensor_scalar(
            out=pe[:, :], in0=p_cur[:, :], scalar1=expEn[:, :],
            scalar2=None, op0=OP.mult)
        z_ps = psC.tile([1, BL], F32, tag="psC")
        nc.tensor.matmul(z_ps[:, :], lhsT=ones9[:, :], rhs=pe[:, :],
                         start=True, stop=True)
        nc.vector.tensor_copy(sall[:, nidx * BL:(nidx + 1) * BL], z_ps[:, :])

        sall_log = cst.tile([1, sall_w], F32, tag="sall_log")
        nc.scalar.activation(sall_log[:, :], sall[:, :], AF.Ln)
        logz = cst.tile([1, BL], F32, tag="logz")
        nc.vector.tensor_reduce(
            out=logz[:, :],
            in_=sall_log.rearrange("p (n b) -> p b n", b=BL),
            axis=mybir.AxisListType.X, op=OP.add)

        # ---------- loss = sum_b (logZ - score) ----------
        diff = cst.tile([1, BL], F32, tag="diff")
        nc.vector.tensor_tensor(out=diff[:, :], in0=logz[:, :],
                                in1=score[:, :], op=OP.subtract)
        total = cst.tile([1, 1], F32, tag="total")
        nc.vector.tensor_reduce(out=total[:, :], in_=diff[:, :],
                                axis=mybir.AxisListType.X, op=OP.add)
        nc.sync.dma_start(out=loss_d[:, :], in_=total[:, :])
        if debug:
            nc.sync.dma_start(out=score_d[:, :], in_=score[:, :])
            nc.sync.dma_start(out=logz_d[:, :], in_=logz[:, :])

    return nc


def host_prep(src_input, labels, embedding, W_ih, W_hh, b_ih, b_hh,
              W_lin, b_lin, start_trans, end_trans, trans,
              n_steps: int = S):
    """Build the 8 per-core input maps."""
    f32 = np.float32
    wihT = np.asarray(W_ih, dtype=f32).T      # [E, 4H]
    whhT = np.asarray(W_hh, dtype=f32).T      # [H, 4H]
    wlinT = np.asarray(W_lin, dtype=f32).T    # [H, L]
    wpack = np.zeros((128, 4114), f32)
    wpack[:, 0:1024] = wihT[0:128]
    wpack[:, 1024:2048] = wihT[128:256]
    wpack[:, 2048:3072] = whhT[0:128]
    wpack[:, 3072:4096] = whhT[128:256]
    wpack[:, 4096:4105] = wlinT[0:128]
    wpack[:, 4105:4114] = wlinT[128:256]
    spack = np.zeros((128, 36), f32)
    spack[:, 0:8] = np.asarray(b_ih, dtype=f32).reshape(8, 128).T
    spack[:, 8:16] = np.asarray(b_hh, dtype=f32).reshape(8, 128).T
    spack[0:L, 16] = np.asarray(start_trans, dtype=f32)
    spack[0:L, 17] = np.asarray(end_trans, dtype=f32)
    spack[0:L, 18:27] = np.asarray(trans, dtype=f32)
    spack[0, 27:36] = np.asarray(b_lin, dtype=f32)
    shared = {
        "emb": np.ascontiguousarray(embedding, dtype=f32),
        "wpack": wpack,
        "spack": spack,
    }
    in_maps = []
    for c in range(NCORES):
        rows = slice(c * BL, (c + 1) * BL)
        src_c = np.asarray(src_input[rows, :n_steps], dtype=np.int32)
        lab_c = np.asarray(labels[rows, :n_steps], dtype=np.int32)
        m = dict(shared)
        m["idx"] = np.ascontiguousarray(src_c.T).reshape(n_steps * BL, 1)
        m["labT"] = np.ascontiguousarray(lab_c.T)
        in_maps.append(m)
    return in_maps


_CACHED = {}


def _get_program(n_steps=S, debug=False):
    key = (n_steps, debug)
    if key not in _CACHED:
        nc = build_program(n_steps, debug)
        nc.finalize()
        _CACHED[key] = nc
    return _CACHED[key]


def kernel(src_input, labels, masks, embedding, W_ih, W_hh, b_ih, b_hh,
           W_lin, b_lin, start_trans, end_trans, trans):
    # masks are all-ones by construction (torchcrf requires mask[:,0]); the
    # kernel hardcodes full-length sequences.
    nc = _get_program(S, debug=False)
    in_maps = host_prep(src_input, labels, embedding, W_ih, W_hh,
                        b_ih, b_hh, W_lin, b_lin, start_trans,
                        end_trans, trans)
    res = run_bass_kernel_spmd(nc, in_maps, core_ids=list(range(NCORES)))
    parts = [res.results[i]["loss"][0, 0] for i in range(NCORES)]
    return np.float32(np.sum(np.asarray(parts, dtype=np.float32)))


# revision 22
# speedup vs baseline: 1.1360x; 1.1360x over previous
"""Bass/Trainium2 kernel for nn_EntityLabeler (LSTM+CRF NLL loss).

Contract: kernel(**inputs) takes FULL unsharded inputs (as produced by
setup_inputs) and returns the FULL scalar loss. Internally shards the
batch (128 rows) across 8 NeuronCores (16 rows each), computes a partial
loss per core on-device, and sums the 8 partials on the host.

Device algorithm per core (all layouts transposed: feature-on-partition,
batch-on-free):
  1. Embedding gather (indirect DMA) in 32-step chunks -> PE transpose ->
     x.T tiles (bf16).
  2. Input projection xp.T = W_ih @ x.T + b (matmul, bf16) into an SBUF
     ring, packed per-step as [i0 i1 f0 f1 o0 o1 g0 g1] x 16 batch.
  3. LSTM recurrence: per step one identity-matmul injects xp into PSUM,
     then 16 accumulate matmuls (W_hh.T stationary, h.T moving) produce
     gates.T [128, 128]; sigmoid/tanh on ACT, cell update on DVE.
  4. Emissions em.T = W_lin @ relu(h.T) + b_lin via matmul (b_lin folded
     in as a K=1 matmul row).
  5. CRF log-partition via exp-domain linear scan:
     p <- (ET.T @ p) * exp(em_t), renormalized every 8 steps;
     logZ = sum(log s) + log(sum p*exp(end)).
  6. Gold-path score via one-hot matmuls (L=9).
"""

import sys
from contextlib import ExitStack

import numpy as np

for _p in ("/opt/trn_rl_repo",):
    if _p not in sys.path:
        sys.path.insert(0, _p)

import concourse.bass as bass
import concourse.bacc as bacc
import concourse.tile as tile
from concourse import mybir
from concourse.masks import make_identity
from concourse.bass_utils import run_bass_kernel_spmd

F32 = mybir.dt.float32
BF16 = mybir.dt.bfloat16
I32 = mybir.dt.int32
AF = mybir.ActivationFunctionType
OP = mybir.AluOpType

B, S, V, E, H, L = 128, 512, 32000, 256, 256, 9
NCORES = 8
BL = B // NCORES           # 16 batch rows per core
G4 = 4 * H                 # 1024 gate units
CH = 32                    # LSTM steps per chunk
NORM_EVERY = 8

# column offset of each (gate, half) region inside the per-step [128, 128]
# gates.T PSUM tile / xp ring block. gate order (torch): i=0, f=1, g=2, o=3.
POS = {(0, 0): 0, (0, 1): 16, (1, 0): 32, (1, 1): 48,
       (3, 0): 64, (3, 1): 80, (2, 0): 96, (2, 1): 112}


def build_program(n_steps: int = S, debug: bool = False):
    """Emit the full Bass/Tile program for one core. Returns nc."""
    assert n_steps % CH == 0
    nchunk = n_steps // CH
    ctok = CH * BL                     # tokens per chunk (512)
    tok = n_steps * BL
    n_norm = (n_steps - 1) // NORM_EVERY       # renormalizations in scan
    sall_w = (n_norm + 1) * BL                 # log-factors incl. final z

    nc = bacc.Bacc("TRN2", target_bir_lowering=False)

    # ---- DRAM I/O ----
    emb_d = nc.dram_tensor("emb", [V, E], F32, kind="ExternalInput")
    idx_d = nc.dram_tensor("idx", [tok, 1], I32, kind="ExternalInput")
    labT_d = nc.dram_tensor("labT", [n_steps, BL], I32, kind="ExternalInput")
    # all weights in one array (single DMA -> single wait for consumers):
    # cols [0:1024] wihT k0, [1024:2048] wihT k1, [2048:3072] whhT k0,
    # [3072:4096] whhT k1, [4096:4105] wlinT k0, [4105:4114] wlinT k1
    wpack_d = nc.dram_tensor("wpack", [128, 4114], F32, kind="ExternalInput")
    # small constants in one array: cols [0:8] bihT, [8:16] bhhT,
    # [16] stT, [17] enT, [18:27] trans, [27:36] blin row (partition 0)
    spack_d = nc.dram_tensor("spack", [128, 36], F32, kind="ExternalInput")

    loss_d = nc.dram_tensor("loss", [1, 1], F32, kind="ExternalOutput")
    if debug:
        score_d = nc.dram_tensor("score", [1, BL], F32, kind="ExternalOutput")
        logz_d = nc.dram_tensor("logz", [1, BL], F32, kind="ExternalOutput")

    with tile.TileContext(nc) as tc, ExitStack() as ctx:
        cst = ctx.enter_context(tc.tile_pool(name="cst", bufs=1))
        stage = ctx.enter_context(tc.tile_pool(name="stage", bufs=2))
        big = ctx.enter_context(tc.tile_pool(name="bigbuf", bufs=1))
        xgp = ctx.enter_context(tc.tile_pool(name="xgp", bufs=6))
        xtp = ctx.enter_context(tc.tile_pool(name="xtp", bufs=4))
        xpr = ctx.enter_context(tc.tile_pool(name="xpr", bufs=2))
        hcp = ctx.enter_context(tc.tile_pool(name="hcp", bufs=3))
        gat = ctx.enter_context(tc.tile_pool(name="gat", bufs=4))
        sml = ctx.enter_context(tc.tile_pool(name="sml", bufs=6))
        scn = ctx.enter_context(tc.tile_pool(name="scn", bufs=6))
        psA = ctx.enter_context(tc.tile_pool(name="psA", bufs=4, space="PSUM"))
        psB = ctx.enter_context(tc.tile_pool(name="psB", bufs=2, space="PSUM"))
        psC = ctx.enter_context(tc.tile_pool(name="psC", bufs=2, space="PSUM"))

        # ---------- constants / weights ----------
        id_bf = cst.tile([128, 128], BF16, tag="id_bf")
        make_identity(nc, id_bf[:, :])
        id_f32 = cst.tile([128, 128], F32, tag="id_f32")
        make_identity(nc, id_f32[:, :])

        warm_ps = psC.tile([1, 1], F32, tag="psC", name="warm_ps")
        nc.tensor.matmul(warm_ps[:, :], lhsT=id_f32[:, 0:1],
                         rhs=id_f32[:, 0:1], start=True, stop=True)

        wpk = cst.tile([128, 4114], F32, tag="wpk")
        nc.sync.dma_start(out=wpk[:, :], in_=wpack_d[:, :])
        spk = cst.tile([128, 36], F32, tag="spk")
        nc.sync.dma_start(out=spk[:, :], in_=spack_d[:, :])

        def cast_bf(src_ap, n_m, tag):
            bf_t = cst.tile([128, n_m], BF16, tag=tag)
            nc.vector.tensor_copy(bf_t[:, :], src_ap)
            return bf_t

        wih_bf = [cast_bf(wpk[:, c * 1024:(c + 1) * 1024], 1024, f"wih{c}")
                  for c in range(2)]
        whh_bf = [cast_bf(wpk[:, 2048 + c * 1024: 2048 + (c + 1) * 1024],
                          1024, f"whh{c}") for c in range(2)]
        wlin_bf = [cast_bf(wpk[:, 4096 + c * L: 4096 + (c + 1) * L], L,
                           f"wlin{c}") for c in range(2)]

        bsum = cst.tile([128, 8], F32, tag="bsum")
        nc.vector.tensor_add(bsum[:, :], spk[:, 0:8], spk[:, 8:16])
        stT = spk[0:L, 16:17]
        enT = spk[0:L, 17:18]
        trans_t = spk[0:L, 18:27]
        blin_bf = cst.tile([1, L], BF16, tag="blinbf")
        nc.vector.tensor_copy(blin_bf[:, :], spk[0:1, 27:36])
        ones_ctok_bf = cst.tile([1, ctok], BF16, tag="onesctok")
        nc.vector.memset(ones_ctok_bf[:, :], 1.0)

        expSt = cst.tile([L, 1], F32, tag="expSt")
        nc.scalar.activation(expSt[:, :], stT, AF.Exp)
        expEn = cst.tile([L, 1], F32, tag="expEn")
        nc.scalar.activation(expEn[:, :], enT, AF.Exp)
        ET = cst.tile([L, L], F32, tag="ET")
        nc.scalar.activation(ET[:, :], trans_t, AF.Exp)
        ones9 = cst.tile([L, 1], F32, tag="ones9")
        nc.vector.memset(ones9[:, :], 1.0)
        ones1_9 = cst.tile([1, L], F32, tag="ones19")
        nc.vector.memset(ones1_9[:, :], 1.0)

        # ---------- one-hot label matrix OHT [L, tok] ----------
        iota9 = cst.tile([L, 1], I32, tag="iota9")
        nc.gpsimd.iota(iota9[:, :], pattern=[[0, 1]], base=0, channel_multiplier=1)
        iota9f = cst.tile([L, 1], F32, tag="iota9f")
        nc.vector.tensor_copy(iota9f[:, :], iota9[:, :])
        OHT = big.tile([L, tok], F32, tag="OHT")
        lab1 = stage.tile([1, tok], I32, tag="lab1", bufs=1)
        lab_flat = bass.AP(tensor=labT_d, offset=0, ap=[[0, 1], [1, tok]])
        nc.sync.dma_start(out=lab1[:, :], in_=lab_flat)
        lchunk = 512
        for q in range(tok // lchunk):
            sl = slice(q * lchunk, (q + 1) * lchunk)
            labf1 = stage.tile([1, lchunk], F32, tag="labf1")
            nc.vector.tensor_copy(labf1[:, :], lab1[:, sl])
            lab_ps = psC.tile([L, lchunk], F32, tag="psC", name="lab_ps")
            nc.tensor.matmul(lab_ps[:, :], lhsT=ones1_9[:, :],
                             rhs=labf1[:, :], start=True, stop=True)
            labrep = stage.tile([L, lchunk], F32, tag="labrep")
            nc.vector.tensor_copy(labrep[:, :], lab_ps[:, :])
            nc.vector.tensor_scalar(
                out=OHT[:, sl], in0=labrep[:, :],
                scalar1=iota9f[:, :], scalar2=None, op0=OP.is_equal)

        # ---------- big persistent buffers ----------
        EE = big.tile([L, tok], F32, tag="EE")          # exp(emissions.T)
        sall = big.tile([1, sall_w], F32, tag="sall")   # scan log-factors
        etsum = cst.tile([1, BL], F32, tag="etsum")     # sum_t em[lab] per b
        nc.vector.memset(etsum[:, :], 0.0)

        # ---------- numerator: transition scores ----------
        trsum = cst.tile([1, BL], F32, tag="trsum")
        nc.vector.memset(trsum[:, :], 0.0)
        for cc in range(nchunk):
            w = ctok if cc < nchunk - 1 else ctok - BL
            q_ps = psB.tile([L, ctok], F32, tag="psB")
            nc.tensor.matmul(
                q_ps[:, :w], lhsT=trans_t[:, :],
                rhs=OHT[:, cc * ctok: cc * ctok + w], start=True, stop=True)
            tprod = stage.tile([L, ctok], F32, tag="tprod")
            nc.vector.tensor_tensor(
                out=tprod[:, :w], in0=q_ps[:, :w],
                in1=OHT[:, cc * ctok + BL: cc * ctok + BL + w], op=OP.mult)
            tr_ps = psC.tile([1, ctok], F32, tag="psC")
            nc.tensor.matmul(tr_ps[:, :w], lhsT=ones9[:, :], rhs=tprod[:, :w],
                             start=True, stop=True)
            trc = sml.tile([1, BL], F32, tag="trc")
            nc.vector.tensor_reduce(
                out=trc[:, :],
                in_=tr_ps[:, :w].rearrange("p (t b) -> p b t", b=BL),
                axis=mybir.AxisListType.X, op=OP.add)
            nc.vector.tensor_tensor(
                out=trsum[:, :], in0=trsum[:, :], in1=trc[:, :], op=OP.add)

        # start / end scores: weights-as-lhsT does mul+colsum in one matmul
        st_ps = psC.tile([1, BL], F32, tag="psC")
        nc.tensor.matmul(st_ps[:, :], lhsT=stT[:, :], rhs=OHT[:, 0:BL],
                         start=True, stop=True)
        en_ps = psC.tile([1, BL], F32, tag="psC")
        nc.tensor.matmul(en_ps[:, :], lhsT=enT[:, :],
                         rhs=OHT[:, tok - BL:tok], start=True, stop=True)
        # start/end sums need SBUF homes before PSUM slots recycle
        sten = cst.tile([1, 2 * BL], F32, tag="sten")
        nc.vector.tensor_copy(sten[:, 0:BL], st_ps[:, :])
        nc.vector.tensor_copy(sten[:, BL:2 * BL], en_ps[:, :])


        # all gather indices in one DMA: idx_all[p, g] = idx[g*128 + p]
        idx_all = cst.tile([128, tok // 128], I32, tag="idx_all")
        idx_ap = bass.AP(tensor=idx_d, offset=0,
                         ap=[[1, 128], [128, tok // 128]])
        nc.sync.dma_start(out=idx_all[:, :], in_=idx_ap)

        # ---------- CRF forward scan (exp domain), interleaved ----------
        scan_state = {"p": None, "nidx": 0, "next_t": 1}

        def emit_scan_init():
            p0 = scn.tile([L, BL], F32, tag="p", name="p_init")
            nc.vector.tensor_scalar(
                out=p0[:, :], in0=EE[:, 0:BL], scalar1=expSt[:, :],
                scalar2=None, op0=OP.mult)
            scan_state["p"] = p0

        def emit_scan_step(t):
            q_ps = psC.tile([L, BL], F32, tag="psC", name="scan_q")
            nc.tensor.matmul(q_ps[:, :], lhsT=ET[:, :],
                             rhs=scan_state["p"][:, :], start=True, stop=True)
            p_new = scn.tile([L, BL], F32, tag="p", name="p_new")
            nc.vector.tensor_tensor(
                out=p_new[:, :], in0=q_ps[:, :],
                in1=EE[:, t * BL:(t + 1) * BL], op=OP.mult)
            scan_state["p"] = p_new
            if t % NORM_EVERY == 0:
                nidx = scan_state["nidx"]
                s_ps = psC.tile([1, BL], F32, tag="psC", name="scan_s")
                nc.tensor.matmul(s_ps[:, :], lhsT=ones9[:, :],
                                 rhs=p_new[:, :], start=True, stop=True)
                nc.vector.tensor_copy(
                    sall[:, nidx * BL:(nidx + 1) * BL], s_ps[:, :])
                rs = scn.tile([1, BL], F32, tag="rs")
                nc.vector.reciprocal(rs[:, :], s_ps[:, :])
                bc_ps = psC.tile([L, BL], F32, tag="psC", name="scan_bc")
                nc.tensor.matmul(bc_ps[:, :], lhsT=ones1_9[:, :],
                                 rhs=rs[:, :], start=True, stop=True)
                p2 = scn.tile([L, BL], F32, tag="p", name="p_norm")
                nc.vector.tensor_tensor(
                    out=p2[:, :], in0=p_new[:, :], in1=bc_ps[:, :],
                    op=OP.mult)
                scan_state["p"] = p2
                scan_state["nidx"] += 1
            scan_state["next_t"] = t + 1

        # ---------- main chunk pipeline ----------
        cstate = cst.tile([128, 32], F32, tag="cstate")  # c.T both halves
        h_prev = None        # AP of previous step's h.T [128, 32] (bf16)
        hT_chunks = []

        for k in range(nchunk):
            # -- gather 512 tokens & transpose to x.T (bf16) --
            xT = [xtp.tile([128, ctok], BF16, tag="xT", name=f"xT{ec}")
                  for ec in range(2)]
            for q in range(4):
                g = k * 4 + q
                xg = xgp.tile([128, E], F32, tag="xg")
                nc.gpsimd.indirect_dma_start(
                    out=xg[:, :], out_offset=None,
                    in_=emb_d[:, :],
                    in_offset=bass.IndirectOffsetOnAxis(
                        ap=idx_all[:, g:g + 1], axis=0))
                for ec in range(2):
                    tp = psA.tile([128, 128], F32, tag="psA")
                    nc.tensor.transpose(
                        tp[:, :], xg[:, ec * 128:(ec + 1) * 128], id_f32[:, :])
                    dst = xT[ec][:, q * 128:(q + 1) * 128]
                    nc.vector.tensor_copy(dst, tp[:, :])

            # -- input projection xp ring for this chunk --
            xpring = xpr.tile([128, CH * 128], BF16, tag="xpring")
            xpv = xpring.rearrange("p (t g) -> p t g", g=128)
            for gi, half in ((0, 0), (0, 1), (1, 0), (1, 1),
                             (3, 0), (3, 1), (2, 0), (2, 1)):
                j = gi * 2 + half
                xp_ps = psB.tile([128, ctok], F32, tag="psB")
                for c in range(2):
                    nc.tensor.matmul(
                        xp_ps[:, :],
                        lhsT=wih_bf[c][:, j * 128:(j + 1) * 128],
                        rhs=xT[c][:, :], start=(c == 0), stop=(c == 1))
                src = xp_ps.rearrange("p (t b) -> p t b", b=BL)
                dst = xpv[:, :, POS[(gi, half)]:POS[(gi, half)] + BL]
                nc.scalar.add(dst, src, add=bsum[:, j:j + 1])

            # sync DVE's view of ACT's xpring writes (keeps every
            # consumer at <=1 semaphore wait; walrus ISA limit)
            sync_j = sml.tile([128, 1], BF16, tag="syncj")
            nc.vector.tensor_copy(sync_j[:, :], xpring[:, 0:1])

            # -- LSTM recurrence over this chunk --
            hT = hcp.tile([128, CH * 32], BF16, tag="hT")
            hT_chunks.append(hT)
            for tl in range(CH):
                t = k * CH + tl
                if t == 0:
                    # h == 0: gates are just the input projection
                    gpre_i = xpv[:, 0, 0:96]
                    gpre_g = xpv[:, 0, 96:128]
                else:
                    ps = psA.tile([128, 96], F32, tag="psA", name="ps_ifo")
                    ps_g = psA.tile([128, 32], F32, tag="psA", name="ps_g")
                    for gi, half in ((2, 0), (2, 1), (0, 0), (0, 1),
                                     (1, 0), (1, 1), (3, 0), (3, 1)):
                        j = gi * 2 + half
                        pos = POS[(gi, half)]
                        dst = (ps_g[:, pos - 96:pos - 96 + BL] if gi == 2
                               else ps[:, pos:pos + BL])
                        for c in range(2):
                            nc.tensor.matmul(
                                dst,
                                lhsT=whh_bf[c][:, j * 128:(j + 1) * 128],
                                rhs=h_prev[:, c * BL:(c + 1) * BL],
                                start=(c == 0), stop=(c == 1))
                    gi_t = gat.tile([128, 96], F32, tag="gprei")
                    nc.vector.tensor_tensor(
                        out=gi_t[:, :], in0=ps[:, :], in1=xpv[:, tl, 0:96],
                        op=OP.add)
                    gg_t = gat.tile([128, 32], F32, tag="gpreg")
                    nc.vector.tensor_tensor(
                        out=gg_t[:, :], in0=ps_g[:, :],
                        in1=xpv[:, tl, 96:128], op=OP.add)
                    gpre_i, gpre_g = gi_t[:, :], gg_t[:, :]
                sifo = gat.tile([128, 96], F32, tag="sifo")
                nc.scalar.activation(sifo[:, :], gpre_i, AF.Sigmoid)
                tg = gat.tile([128, 32], F32, tag="tg")
                nc.scalar.activation(tg[:, :], gpre_g, AF.Tanh)
                if t == 0:
                    nc.vector.tensor_tensor(
                        out=cstate[:, :], in0=sifo[:, 0:32], in1=tg[:, :],
                        op=OP.mult)
                else:
                    fc = sml.tile([128, 32], F32, tag="fc")
                    nc.vector.tensor_tensor(
                        out=fc[:, :], in0=sifo[:, 32:64], in1=cstate[:, :],
                        op=OP.mult)
                    ig = sml.tile([128, 32], F32, tag="ig")
                    nc.vector.tensor_tensor(
                        out=ig[:, :], in0=sifo[:, 0:32], in1=tg[:, :],
                        op=OP.mult)
                    nc.vector.tensor_tensor(
                        out=cstate[:, :], in0=fc[:, :], in1=ig[:, :],
                        op=OP.add)
                tc_t = gat.tile([128, 32], F32, tag="tc")
                nc.scalar.activation(tc_t[:, :], cstate[:, :], AF.Tanh)
                h_slice = hT[:, tl * 32:(tl + 1) * 32]
                nc.vector.tensor_tensor(
                    out=h_slice, in0=sifo[:, 64:96], in1=tc_t[:, :],
                    op=OP.mult)
                h_prev = h_slice

            # -- emissions for this chunk --
            relu_t = xtp.tile([128, CH * 32], BF16, tag="relu")
            nc.scalar.activation(relu_t[:, :], hT[:, :], AF.Relu)
            rv = relu_t.rearrange("p (t s) -> p t s", s=32)
            em_ps = psB.tile([L, ctok], F32, tag="psB")
            for c in range(2):
                nc.tensor.matmul(
                    em_ps[:, :], lhsT=wlin_bf[c][:, :],
                    rhs=rv[:, :, c * BL:(c + 1) * BL],
                    start=(c == 0), stop=False)
            nc.tensor.matmul(
                em_ps[:, :], lhsT=blin_bf[:, :], rhs=ones_ctok_bf[:, :],
                start=False, stop=True)
            nc.scalar.activation(
                EE[:, k * ctok:(k + 1) * ctok], em_ps[:, :], AF.Exp)
            prod = stage.tile([L, ctok], F32, tag="prod")
            nc.vector.tensor_tensor(
                out=prod[:, :], in0=em_ps[:, :],
                in1=OHT[:, k * ctok:(k + 1) * ctok], op=OP.mult)
            et_ps = psC.tile([1, ctok], F32, tag="psC")
            nc.tensor.matmul(et_ps[:, :], lhsT=ones9[:, :], rhs=prod[:, :],
                             start=True, stop=True)
            etc = sml.tile([1, BL], F32, tag="etc")
            nc.vector.tensor_reduce(
                out=etc[:, :], in_=et_ps.rearrange("p (t b) -> p b t", b=BL),
                axis=mybir.AxisListType.X, op=OP.add)
            nc.vector.tensor_tensor(
                out=etsum[:, :], in0=etsum[:, :], in1=etc[:, :], op=OP.add)

            if k == 0:
                emit_scan_init()
            for t in range(scan_state["next_t"], (k + 1) * CH):
                emit_scan_step(t)

        score = cst.tile([1, BL], F32, tag="score")
        nc.vector.tensor_copy(score[:, :], sten[:, 0:BL])
        nc.vector.tensor_tensor(out=score[:, :], in0=score[:, :],
                                in1=sten[:, BL:2 * BL], op=OP.add)
        nc.vector.tensor_tensor(out=score[:, :], in0=score[:, :],
                                in1=etsum[:, :], op=OP.add)
        nc.vector.tensor_tensor(out=score[:, :], in0=score[:, :],
                                in1=trsum[:, :], op=OP.add)

        # ---------- CRF forward scan: remaining steps ----------
        for t in range(scan_state["next_t"], n_steps):
            emit_scan_step(t)
        pe = scn.tile([L, BL], F32, tag="pe")
        nc.vector.tensor_scalar(
            out=pe[:, :], in0=scan_state["p"][:, :], scalar1=expEn[:, :],
            scalar2=None, op0=OP.mult)
        z_ps = psC.tile([1, BL], F32, tag="psC")
        nc.tensor.matmul(z_ps[:, :], lhsT=ones9[:, :], rhs=pe[:, :],
                         start=True, stop=True)
        nc.vector.tensor_copy(sall[:, scan_state["nidx"] * BL:(scan_state["nidx"] + 1) * BL], z_ps[:, :])

        sall_log = cst.tile([1, sall_w], F32, tag="sall_log")
        nc.scalar.activation(sall_log[:, :], sall[:, :], AF.Ln)
        logz = cst.tile([1, BL], F32, tag="logz")
        nc.vector.tensor_reduce(
            out=logz[:, :],
            in_=sall_log.rearrange("p (n b) -> p b n", b=BL),
            axis=mybir.AxisListType.X, op=OP.add)

        # ---------- loss = sum_b (logZ - score) ----------
        diff = cst.tile([1, BL], F32, tag="diff")
        nc.vector.tensor_tensor(out=diff[:, :], in0=logz[:, :],
                                in1=score[:, :], op=OP.subtract)
        total = cst.tile([1, 1], F32, tag="total")
        nc.vector.tensor_reduce(out=total[:, :], in_=diff[:, :],
                                axis=mybir.AxisListType.X, op=OP.add)
        nc.sync.dma_start(out=loss_d[:, :], in_=total[:, :])
        if debug:
            nc.sync.dma_start(out=score_d[:, :], in_=score[:, :])
            nc.sync.dma_start(out=logz_d[:, :], in_=logz[:, :])

    return nc


def host_prep(src_input, labels, embedding, W_ih, W_hh, b_ih, b_hh,
              W_lin, b_lin, start_trans, end_trans, trans,
              n_steps: int = S):
    """Build the 8 per-core input maps."""
    f32 = np.float32
    wihT = np.asarray(W_ih, dtype=f32).T      # [E, 4H]
    whhT = np.asarray(W_hh, dtype=f32).T      # [H, 4H]
    wlinT = np.asarray(W_lin, dtype=f32).T    # [H, L]
    wpack = np.zeros((128, 4114), f32)
    wpack[:, 0:1024] = wihT[0:128]
    wpack[:, 1024:2048] = wihT[128:256]
    wpack[:, 2048:3072] = whhT[0:128]
    wpack[:, 3072:4096] = whhT[128:256]
    wpack[:, 4096:4105] = wlinT[0:128]
    wpack[:, 4105:4114] = wlinT[128:256]
    spack = np.zeros((128, 36), f32)
    spack[:, 0:8] = np.asarray(b_ih, dtype=f32).reshape(8, 128).T
    spack[:, 8:16] = np.asarray(b_hh, dtype=f32).reshape(8, 128).T
    spack[0:L, 16] = np.asarray(start_trans, dtype=f32)
    spack[0:L, 17] = np.asarray(end_trans, dtype=f32)
    spack[0:L, 18:27] = np.asarray(trans, dtype=f32)
    spack[0, 27:36] = np.asarray(b_lin, dtype=f32)
    shared = {
        "emb": np.ascontiguousarray(embedding, dtype=f32),
        "wpack": wpack,
        "spack": spack,
    }
    in_maps = []
    for c in range(NCORES):
        rows = slice(c * BL, (c + 1) * BL)
        src_c = np.asarray(src_input[rows, :n_steps], dtype=np.int32)
        lab_c = np.asarray(labels[rows, :n_steps], dtype=np.int32)
        m = dict(shared)
        m["idx"] = np.ascontiguousarray(src_c.T).reshape(n_steps * BL, 1)
        m["labT"] = np.ascontiguousarray(lab_c.T)
        in_maps.append(m)
    return in_maps


_CACHED = {}


def _get_program(n_steps=S, debug=False):
    key = (n_steps, debug)
    if key not in _CACHED:
        nc = build_program(n_steps, debug)
        nc.finalize()
        _CACHED[key] = nc
    return _CACHED[key]


def kernel(src_input, labels, masks, embedding, W_ih, W_hh, b_ih, b_hh,
           W_lin, b_lin, start_trans, end_trans, trans):
    # masks are all-ones by construction (torchcrf requires mask[:,0]); the
    # kernel hardcodes full-length sequences.
    nc = _get_program(S, debug=False)
    in_maps = host_prep(src_input, labels, embedding, W_ih, W_hh,
                        b_ih, b_hh, W_lin, b_lin, start_trans,
                        end_trans, trans)
    res = run_bass_kernel_spmd(nc, in_maps, core_ids=list(range(NCORES)))
    parts = [res.results[i]["loss"][0, 0] for i in range(NCORES)]
    return np.float32(np.sum(np.asarray(parts, dtype=np.float32)))
